# revision 16
# baseline (speedup 1.0000x reference)
"""Trainium2 Bass kernel for nn_DecoderLayer_44263932953096 (MQA + top-2 MoE).

Strategy (8 NeuronCores, SPMD via run_bass_kernel_spmd, no on-device collectives):
  L1: token-sharded attention + router. Each core computes shared K/V from the
      full sequence, attention for its 256-token shard, post-attn residual +
      rmsnorms and router softmax. Activations flow feature-on-partition
      ("T layout") so every matmul contracts over the partition dim.
  host: top-2 expert selection (integer compare/indexing only), gathers each
      expert's tokens into per-expert batches.
  L2: expert FFN, sliced along the hidden (FH) axis: core c holds the 512-wide
      FH slice c of ALL 8 experts' W1/W2 and processes every expert's token
      batch for its slice. Perfectly load-balanced against routing skew; one
      uniform NEFF (strip token-caps baked per compile, keyed by counts).
  host: permutes per-slice partial outputs back to token order (movement only).
  L3: token-sharded combine: sums the 16 partials (8 FH slices x 2 experts)
      per token, final rmsnorm + residual.

All floating-point arithmetic runs on device; the host only transposes,
slices, gathers, packs and permutes. Matmuls run as float32r (full-rate fp32).
Every DRAM tensor is host-packed into the exact [128, nchunk*width] big-tile
SBUF layout (chunk-major columns) so each load/store is one long-run 2D DMA.
"""

from contextlib import ExitStack

import numpy as np

import concourse.bacc as bacc
import concourse.mybir as mybir
from concourse import tile
from concourse.bass_utils import run_bass_kernel_spmd

F32 = mybir.dt.float32
F32R = mybir.dt.float32r
AF = mybir.ActivationFunctionType
OP = mybir.AluOpType
AX = mybir.AxisListType

S, D, H, HD, E, TOPK, FH = 2048, 1024, 16, 64, 8, 2, 4096
NCORES = 8
SH = S // NCORES          # 256 tokens per shard
DC = D // 128             # 8 feature chunks
FSLICE = FH // NCORES     # 512 hidden units per core slice
EPS = 1e-5
CORE_IDS = list(range(NCORES))

_cache = {}
L2_BF16 = False


# ---------------------------------------------------------------- helpers

def _nc():
    return bacc.Bacc("TRN2", target_bir_lowering=False, debug=False,
                     num_devices=NCORES)


def _chunks(n, lim=512):
    assert n % 2 == 0
    k = (n + lim - 1) // lim
    base = (n // k) & ~1
    sizes = [base] * k
    deficit = n - base * k
    i = 0
    while deficit > 0:
        sizes[i] += 2; deficit -= 2; i = (i + 1) % k
    return sizes


def pack(m):
    """[nc*128, w] -> big-tile layout [128, nc*w] (chunk-major columns)."""
    nc_, w = m.shape[0] // 128, m.shape[1]
    return np.ascontiguousarray(
        m.reshape(nc_, 128, w).transpose(1, 0, 2).reshape(128, nc_ * w))


def unpack(p, nchunk):
    """inverse of pack: [128, nc*w] -> [nc*128, w]."""
    w = p.shape[1] // nchunk
    return np.ascontiguousarray(
        p.reshape(128, nchunk, w).transpose(1, 0, 2).reshape(nchunk * 128, w))


def _rmsnorm_T(nc, ps_small, big_in, g_big, ones_k, eps_t, N, pool, tag,
               out_bufs=1, out_dt=F32R):
    """T-layout rmsnorm on a big tile [128, DC*N] -> big tile [128, DC*N]:
    out = in * rsqrt(mean_over_D(in^2) + eps) * g.
    g_big is a [128, DC]-shaped view (column kc = gamma chunk kc)."""
    inv = pool.tile([1, N], F32, name=f"{tag}_inv", tag=f"{tag}_inv", bufs=2)
    off = 0
    chm = _chunks(N)[0]
    for ch in _chunks(N):
        pss = ps_small.tile([1, 512], F32, name="colsum", tag="small", bufs=2)
        sq = pool.tile([128, DC * chm], F32R, name=f"{tag}_sq", tag=f"{tag}_sq",
                       bufs=2)
        for kc in range(DC):
            s = sq[:, kc * chm:kc * chm + ch]
            nc.scalar.square(s, big_in[:, kc * N + off:kc * N + off + ch])
            nc.tensor.matmul(pss[:, :ch], ones_k[:], s,
                             start=(kc == 0), stop=(kc == DC - 1))
        nc.scalar.activation(inv[:, off:off + ch], pss[:, :ch], AF.Sqrt,
                             bias=eps_t[:], scale=1.0 / D)
        off += ch
    nc.vector.reciprocal(inv[:], inv[:])
    inv_b = pool.tile([128, N], F32, name=f"{tag}_invb", tag=f"{tag}_invb",
                      bufs=2)
    nc.gpsimd.partition_broadcast(inv_b[:], inv[:])
    out = pool.tile([128, DC * N], out_dt, name=f"{tag}_o", tag=f"{tag}_o",
                    bufs=out_bufs)
    for kc in range(DC):
        nc.vector.scalar_tensor_tensor(out[:, kc * N:(kc + 1) * N],
                                       big_in[:, kc * N:(kc + 1) * N],
                                       g_big[:, kc:kc + 1], inv_b[:],
                                       OP.mult, OP.mult)
    return out


# ---------------------------------------------------------------- L1

def build_l1():
    nc = _nc()
    ap = {}
    def din(name, shape):
        ap[name] = nc.dram_tensor(name, shape, F32, kind="ExternalInput").ap()
    def dout(name, shape):
        ap[name] = nc.dram_tensor(name, shape, F32, kind="ExternalOutput").ap()

    NB = S // 512   # 4 column blocks for the full-sequence pass
    GH = 2          # heads per attention group
    NG = H // GH    # 8 groups
    KC16 = S // 128 # 16 key chunks

    din("xTb", [128, NB * DC * 512])   # packed (block, chunk, col)
    din("xTs", [128, DC * SH])
    din("Wq", [128, DC * D]); din("Wkv", [128, DC * 128])
    din("Wo", [128, DC * D]); din("Wr", [128, DC * E])
    din("gs", [128, DC * 3])
    din("ones", [128]); din("ident", [128, 128])
    dout("x2T", [128, DC * SH]); dout("h2T", [128, DC * SH])
    dout("probs", [SH, E])

    with tile.TileContext(nc) as tc, ExitStack() as ctx:
        keep = ctx.enter_context(tc.tile_pool(name="keep", bufs=1))
        ps_mm = ctx.enter_context(tc.tile_pool(name="ps_mm", bufs=2, space="PSUM"))
        ps_sc = ctx.enter_context(tc.tile_pool(name="ps_sc", bufs=2, space="PSUM"))
        ps_at = ctx.enter_context(tc.tile_pool(name="ps_at", bufs=2, space="PSUM"))
        ps_sm = ctx.enter_context(tc.tile_pool(name="ps_sm", bufs=2, space="PSUM"))

        ones_k = keep.tile([128, 1], F32R, name="ones_k", tag="ones_k")
        nc.sync.dma_start(ones_k[:], ap["ones"].rearrange("(a b) -> a b", b=1).bitcast(F32R))
        eps_t = keep.tile([1, 1], F32, name="eps_t", tag="eps_t")
        nc.vector.memset(eps_t[:], EPS)
        ident = keep.tile([128, 128], F32R, name="ident", tag="ident")
        nc.sync.dma_start(ident[:], ap["ident"].bitcast(F32R))
        gs = keep.tile([128, DC * 3], F32, name="gs", tag="gs")
        nc.sync.dma_start(gs[:], ap["gs"])

        def g_col(which):  # 0=att, 1=post, 2=moe -> [128, DC] strided view
            return gs[:, which::3]

        wr = keep.tile([128, DC * E], F32R, name="wr", tag="wr")
        nc.sync.dma_start(wr[:], ap["Wr"].bitcast(F32R))

        kvT = keep.tile([128, S], F32R, name="kvT", tag="kvT")
        xs = keep.tile([128, DC * SH], F32, name="xs", tag="xs")
        nc.sync.dma_start(xs[:], ap["xTs"])
        att = keep.tile([128, DC * SH], F32R, name="att", tag="att")
        qp = [keep.tile([64, 2 * SH], F32R, name=f"qp{g}", tag=f"qp{g}")
              for g in range(H // 2)]

        with tc.tile_pool(name="stream", bufs=1) as st:
            wq = st.tile([128, DC * D], F32R, name="wq", tag="wq")
            nc.sync.dma_start(wq[:], ap["Wq"].bitcast(F32R))
            wkv = st.tile([128, DC * 128], F32R, name="wkv", tag="wkv")
            nc.sync.dma_start(wkv[:], ap["Wkv"].bitcast(F32R))

            # ---- shard h and per-head qT first (independent of the blocks)
            hs = _rmsnorm_T(nc, ps_sm, xs, g_col(0), ones_k, eps_t, SH, st,
                            tag="sn", out_bufs=2)
            for mc in range(DC):
                pq = ps_mm.tile([128, SH], F32, name="pq", tag="mm")
                for kc in range(DC):
                    nc.tensor.matmul(pq[:], wq[:, kc * D + mc * 128:
                                               kc * D + (mc + 1) * 128],
                                     hs[:, kc * SH:(kc + 1) * SH],
                                     start=(kc == 0), stop=(kc == DC - 1))
                nc.vector.tensor_copy(qp[mc][:, 0:SH], pq[0:64, :])
                nc.vector.tensor_copy(qp[mc][:, SH:2 * SH], pq[64:128, :])

            # ---- full-seq h = rmsnorm(x, g_att), kvT = Wkv.T @ hT
            for nb in range(NB):
                c0 = nb * 512
                xb = st.tile([128, DC * 512], F32, name="xb", tag="xb", bufs=2)
                nc.sync.dma_start(xb[:], ap["xTb"][:, nb * DC * 512:
                                                   (nb + 1) * DC * 512])
                hb = _rmsnorm_T(nc, ps_sm, xb, g_col(0), ones_k, eps_t, 512,
                                st, tag="sn", out_bufs=2)
                pkv = ps_mm.tile([128, 512], F32, name="pkv", tag="mm")
                for kc in range(DC):
                    nc.tensor.matmul(pkv[:], wkv[:, kc * 128:(kc + 1) * 128],
                                     hb[:, kc * 512:(kc + 1) * 512],
                                     start=(kc == 0), stop=(kc == DC - 1))
                nc.vector.tensor_copy(kvT[:, c0:c0 + 512], pkv[:])

        # ---- attention (Wo prefetches concurrently in its own pool)
        wop = ctx.enter_context(tc.tile_pool(name="wop", bufs=1))
        wo = wop.tile([128, DC * D], F32R, name="wo", tag="wo")
        nc.sync.dma_start(wo[:], ap["Wo"].bitcast(F32R))
        with tc.tile_pool(name="attn", bufs=1) as at:
            v_aug = []
            for kc in range(KC16):
                pt = ps_sm.tile([128, 64], F32R, name="pt", tag="small", bufs=2)
                nc.tensor.transpose(pt[:], kvT[64:128, kc * 128:(kc + 1) * 128],
                                    ident[64:128, 0:64])
                va = at.tile([128, 65], F32R, name=f"vaug{kc}", tag=f"vaug{kc}")
                nc.vector.tensor_copy(va[:, 0:64], pt[:])
                nc.vector.tensor_copy(va[:, 64:65], ones_k[:])
                v_aug.append(va)

            for g in range(NG):
                pa = ps_at.tile([65, GH * SH], F32, name="pa", tag="attn")
                for kc in range(KC16):
                    psc = ps_sc.tile([128, GH * SH], F32, name="psc", tag="sc")
                    nc.tensor.matmul(psc[:], kvT[0:64, kc * 128:(kc + 1) * 128],
                                     qp[g][:], start=True, stop=True)
                    ex = at.tile([128, GH * SH], F32R, name="ex", tag="exp",
                                 bufs=16)
                    nc.scalar.activation(ex[:], psc[:], AF.Exp, scale=0.125)
                    nc.tensor.matmul(pa[:], v_aug[kc][:], ex[:],
                                     start=(kc == 0), stop=(kc == KC16 - 1))
                rec = at.tile([1, GH * SH], F32, name="rec", tag="rec", bufs=2)
                nc.vector.reciprocal(rec[:], pa[64:65, :])
                for j in range(GH):
                    h = g * GH + j
                    rb = at.tile([64, SH], F32, name="rb", tag="rb", bufs=2)
                    nc.gpsimd.partition_broadcast(rb[:], rec[:, j * SH:(j + 1) * SH])
                    dst = att[64 * (h % 2):64 * (h % 2) + 64,
                              (h // 2) * SH:(h // 2 + 1) * SH]
                    nc.vector.tensor_tensor(dst, pa[0:64, j * SH:(j + 1) * SH],
                                            rb[:], OP.mult)

        # ---- tail: out-proj, norms, residual, router
        with tc.tile_pool(name="tail", bufs=1) as tl:
            ao = tl.tile([128, DC * SH], F32, name="ao", tag="ao")
            for mc in range(DC):
                po = ps_mm.tile([128, SH], F32, name="po", tag="mm")
                for kc in range(DC):
                    nc.tensor.matmul(po[:], wo[:, kc * D + mc * 128:
                                               kc * D + (mc + 1) * 128],
                                     att[:, kc * SH:(kc + 1) * SH],
                                     start=(kc == 0), stop=(kc == DC - 1))
                nc.vector.tensor_copy(ao[:, mc * SH:(mc + 1) * SH], po[:])

            aon = _rmsnorm_T(nc, ps_sm, ao, g_col(1), ones_k, eps_t, SH, tl,
                             tag="aon", out_dt=F32)
            x2 = tl.tile([128, DC * SH], F32, name="x2", tag="x2")
            for kc in range(DC):
                nc.vector.tensor_tensor(x2[:, kc * SH:(kc + 1) * SH],
                                        xs[:, kc * SH:(kc + 1) * SH],
                                        aon[:, kc * SH:(kc + 1) * SH], OP.add)
            nc.sync.dma_start(ap["x2T"], x2[:])
            h2 = _rmsnorm_T(nc, ps_sm, x2, g_col(2), ones_k, eps_t, SH, tl,
                            tag="h2")
            nc.sync.dma_start(ap["h2T"].bitcast(F32R), h2[:])

            for t2 in range(SH // 128):
                pl = ps_sm.tile([128, E], F32, name="pl", tag="small", bufs=2)
                for kc in range(DC):
                    nc.tensor.matmul(pl[:], h2[:, kc * SH + t2 * 128:
                                               kc * SH + (t2 + 1) * 128],
                                     wr[:, kc * E:(kc + 1) * E],
                                     start=(kc == 0), stop=(kc == DC - 1))
                mx = tl.tile([128, 1], F32, name="mx", tag="mx", bufs=2)
                nc.vector.reduce_max(mx[:], pl[:], axis=AX.X)
                shl = tl.tile([128, E], F32, name="shl", tag="shl", bufs=2)
                nc.vector.tensor_scalar(shl[:], pl[:], mx[:], None, OP.subtract)
                exl = tl.tile([128, E], F32, name="exl", tag="exl", bufs=2)
                sm = tl.tile([128, 1], F32, name="sm", tag="sm", bufs=2)
                nc.scalar.activation(exl[:], shl[:], AF.Exp, accum_out=sm[:])
                nc.vector.reciprocal(sm[:], sm[:])
                pr = tl.tile([128, E], F32, name="pr", tag="pr", bufs=2)
                nc.vector.tensor_scalar(pr[:], exl[:], sm[:], None, OP.mult)
                nc.sync.dma_start(ap["probs"][t2 * 128:(t2 + 1) * 128, :], pr[:])

    nc.compile()
    return nc


# ---------------------------------------------------------------- L2

def build_l2(caps, bf16=False):
    nc = _nc()
    idt = mybir.dt.bfloat16 if bf16 else F32
    mdt = mybir.dt.bfloat16 if bf16 else F32R
    ap = {}
    for e in range(E):
        c = caps[e]
        ap[f"hg{e}"] = nc.dram_tensor(f"hg{e}", [128, DC * c], idt, kind="ExternalInput").ap()
        ap[f"w1_{e}"] = nc.dram_tensor(f"w1_{e}", [128, DC * FSLICE], idt, kind="ExternalInput").ap()
        ap[f"w2_{e}"] = nc.dram_tensor(f"w2_{e}", [128, (FSLICE // 128) * D], idt, kind="ExternalInput").ap()
        ap[f"wv{e}"] = nc.dram_tensor(f"wv{e}", [1, c], F32, kind="ExternalInput").ap()
        ap[f"y{e}"] = nc.dram_tensor(f"y{e}", [128, DC * c], F32, kind="ExternalOutput").ap()

    MC1 = FSLICE // 128   # 4
    with tile.TileContext(nc) as tc:
        with tc.tile_pool(name="hg", bufs=2) as hgp, \
             tc.tile_pool(name="w", bufs=1) as wp, \
             tc.tile_pool(name="hid", bufs=2) as hp, \
             tc.tile_pool(name="out", bufs=2) as op_, \
             tc.tile_pool(name="ps", bufs=8, space="PSUM") as ps:

            for e in range(E):
                cap = caps[e]
                nch = _chunks(cap)
                w1 = wp.tile([128, DC * FSLICE], mdt, name="w1", tag="w1")
                nc.sync.dma_start(w1[:], ap[f"w1_{e}"] if bf16 else ap[f"w1_{e}"].bitcast(F32R))
                w2 = wp.tile([128, MC1 * D], mdt, name="w2", tag="w2")
                nc.sync.dma_start(w2[:], ap[f"w2_{e}"] if bf16 else ap[f"w2_{e}"].bitcast(F32R))
                hg = hgp.tile([128, DC * cap], mdt, name="hg", tag="hg")
                nc.sync.dma_start(hg[:], ap[f"hg{e}"] if bf16 else ap[f"hg{e}"].bitcast(F32R))
                wvs = op_.tile([1, cap], F32, name="wvs", tag="wvs")
                nc.sync.dma_start(wvs[:], ap[f"wv{e}"])
                wb = op_.tile([128, cap], F32, name="wb", tag="wb")
                nc.gpsimd.partition_broadcast(wb[:], wvs[:])

                hid = hp.tile([128, MC1 * cap], mdt, name="hid", tag="hid")
                off = 0
                for ch in nch:
                    for mc in range(MC1):
                        p = ps.tile([128, 512], F32, name="p1", tag="ps")
                        for kc in range(DC):
                            nc.tensor.matmul(
                                p[:, :ch],
                                w1[:, kc * FSLICE + mc * 128:
                                   kc * FSLICE + (mc + 1) * 128],
                                hg[:, kc * cap + off:kc * cap + off + ch],
                                start=(kc == 0), stop=(kc == DC - 1))
                        nc.scalar.activation(hid[:, mc * cap + off:
                                                 mc * cap + off + ch],
                                             p[:, :ch], AF.Gelu_apprx_tanh)
                    off += ch
                yt = op_.tile([128, DC * cap], F32, name="yt", tag="yt", bufs=1)
                off = 0
                for ch in nch:
                    for mc in range(DC):
                        p = ps.tile([128, 512], F32, name="p2", tag="ps")
                        for kc in range(MC1):
                            nc.tensor.matmul(
                                p[:, :ch],
                                w2[:, kc * D + mc * 128:kc * D + (mc + 1) * 128],
                                hid[:, kc * cap + off:kc * cap + off + ch],
                                start=(kc == 0), stop=(kc == MC1 - 1))
                        nc.vector.tensor_tensor(yt[:, mc * cap + off:
                                                   mc * cap + off + ch],
                                                p[:, :ch],
                                                wb[:, off:off + ch], OP.mult)
                    off += ch
                nc.sync.dma_start(ap[f"y{e}"], yt[:])

    nc.compile()
    return nc


# ---------------------------------------------------------------- L3

NPART = NCORES * TOPK  # 16 partial inputs per token

def build_l3():
    nc = _nc()
    ap = {}
    for p in range(NPART):
        ap[f"p{p}"] = nc.dram_tensor(f"p{p}", [128, DC * SH], F32, kind="ExternalInput").ap()
    ap["x2T"] = nc.dram_tensor("x2T", [128, DC * SH], F32, kind="ExternalInput").ap()
    ap["g"] = nc.dram_tensor("g", [D], F32, kind="ExternalInput").ap()
    ap["ones"] = nc.dram_tensor("ones", [128], F32, kind="ExternalInput").ap()
    ap["x3T"] = nc.dram_tensor("x3T", [128, DC * SH], F32, kind="ExternalOutput").ap()

    with tile.TileContext(nc) as tc:
        with tc.tile_pool(name="sb", bufs=1) as sb, \
             tc.tile_pool(name="st", bufs=1) as st, \
             tc.tile_pool(name="ps", bufs=2, space="PSUM") as ps1:
            ones_k = sb.tile([128, 1], F32R, name="ones_k", tag="ones_k")
            nc.sync.dma_start(ones_k[:], ap["ones"].rearrange("(a b) -> a b", b=1).bitcast(F32R))
            eps_t = sb.tile([1, 1], F32, name="eps_t", tag="eps_t")
            nc.vector.memset(eps_t[:], EPS)
            g_big = sb.tile([128, DC], F32, name="g_big", tag="g_big")
            nc.sync.dma_start(g_big[:], ap["g"].rearrange("(c p) -> p c", p=128))
            x2_t = sb.tile([128, DC * SH], F32, name="x2t", tag="x2t")
            nc.sync.dma_start(x2_t[:], ap["x2T"])

            pins = []
            for p in range(NPART):
                t = st.tile([128, DC * SH], F32, name=f"pin{p}", tag="pin",
                            bufs=6)
                nc.sync.dma_start(t[:], ap[f"p{p}"])
                pins.append(t)
            lvl = pins
            li = 0
            while len(lvl) > 1:
                nxt = []
                for i in range(0, len(lvl), 2):
                    o = st.tile([128, DC * SH], F32, name=f"sum{li}_{i}",
                                tag=f"sum{li}", bufs=(2 if len(lvl) > 2 else 1))
                    nc.vector.tensor_tensor(o[:], lvl[i][:], lvl[i + 1][:],
                                            OP.add)
                    nxt.append(o)
                lvl = nxt
                li += 1
            mo = lvl[0]

            mon = _rmsnorm_T(nc, ps1, mo, g_big, ones_k, eps_t, SH, sb,
                             tag="mon", out_dt=F32)
            x3 = sb.tile([128, DC * SH], F32, name="x3", tag="x3")
            for kc in range(DC):
                nc.vector.tensor_tensor(x3[:, kc * SH:(kc + 1) * SH],
                                        x2_t[:, kc * SH:(kc + 1) * SH],
                                        mon[:, kc * SH:(kc + 1) * SH], OP.add)
            nc.sync.dma_start(ap["x3T"], x3[:])

    nc.compile()
    return nc


# ---------------------------------------------------------------- host

def _run(nc, in_maps):
    return run_bass_kernel_spmd(nc, in_maps, CORE_IDS).results


def kernel(x, Wq, Wk, Wv, Wo, g_pre_mqa, g_post_mqa, g_pre_moe, g_post_moe,
           Wr, W1, W2):
    x = np.ascontiguousarray(np.asarray(x, dtype=np.float32))
    xT = np.ascontiguousarray(x[0].T)                       # [D, S]
    # xTb: packed (block, chunk, col) so each 512-col block is contiguous
    xTb = np.ascontiguousarray(
        xT.reshape(DC, 128, S // 512, 512).transpose(1, 2, 0, 3)
          .reshape(128, (S // 512) * DC * 512))
    Wkv = np.concatenate([np.asarray(Wk), np.asarray(Wv)], axis=1).astype(np.float32)
    ones = np.ones(128, np.float32)
    ident = np.zeros((128, 128), np.float32)
    ident[0:64, 0:64] = np.eye(64)
    ident[64:128, 0:64] = np.eye(64)
    gs = np.stack([np.asarray(g_pre_mqa), np.asarray(g_post_mqa),
                   np.asarray(g_pre_moe)], axis=1).astype(np.float32)  # [D,3]

    if "l1" not in _cache:
        _cache["l1"] = build_l1()
    wq_p = pack(np.asarray(Wq, np.float32))
    wkv_p = pack(Wkv)
    wo_p = pack(np.asarray(Wo, np.float32))
    wr_p = pack(np.asarray(Wr, np.float32))
    gs_p = pack(gs)
    xT_p = pack(xT)                                          # [128, DC*S]
    l1_maps = []
    for c in CORE_IDS:
        xs_p = np.ascontiguousarray(
            xT_p.reshape(128, DC, S)[:, :, c * SH:(c + 1) * SH]
                .reshape(128, DC * SH))
        l1_maps.append({
            "xTb": xTb, "xTs": xs_p, "Wq": wq_p, "Wkv": wkv_p,
            "Wo": wo_p, "Wr": wr_p, "gs": gs_p, "ones": ones, "ident": ident,
        })
    r1 = _run(_cache["l1"], l1_maps)
    # packed per-shard [128, DC*SH] -> h2 full matrix [D, S] for gathering
    h2_pk = np.stack([r1[c]["h2T"].reshape(128, DC, SH) for c in CORE_IDS])
    h2_full = np.ascontiguousarray(
        h2_pk.transpose(1, 2, 0, 3).reshape(128, DC, S))     # [128, DC, S]
    probs = np.concatenate([r1[c]["probs"] for c in CORE_IDS], axis=0)  # [S, E]

    # ---- host routing bookkeeping (indexing only)
    order = np.argsort(-probs, axis=1, kind="stable")
    top2 = order[:, :TOPK]                                   # [S, 2]
    idx_e = [np.where((top2 == e).any(axis=1))[0] for e in range(E)]
    counts = [len(ix) for ix in idx_e]
    caps = tuple(max(256, -(-cnt // 64) * 64) for cnt in counts)

    key = ("l2", caps, L2_BF16)
    if key not in _cache:
        _cache[key] = build_l2(caps, bf16=L2_BF16)

    W1 = np.asarray(W1, np.float32); W2 = np.asarray(W2, np.float32)
    if L2_BF16:
        import ml_dtypes
        idt = ml_dtypes.bfloat16
    else:
        idt = np.float32
    l2_shared = {}
    for e in range(E):
        cap, ix = caps[e], idx_e[e]
        hg = np.zeros((128, DC, cap), np.float32)
        hg[:, :, :len(ix)] = h2_full[:, :, ix]
        l2_shared[f"hg{e}"] = hg.reshape(128, DC * cap).astype(idt)
        wv = np.zeros((1, cap), np.float32)
        wv[0, :len(ix)] = probs[ix, e]
        l2_shared[f"wv{e}"] = wv
    l2_maps = []
    for c in CORE_IDS:
        m = dict(l2_shared)
        for e in range(E):
            m[f"w1_{e}"] = pack(np.ascontiguousarray(
                W1[e][:, c * FSLICE:(c + 1) * FSLICE])).astype(idt)
            m[f"w2_{e}"] = pack(np.ascontiguousarray(
                W2[e][c * FSLICE:(c + 1) * FSLICE, :])).astype(idt)
        l2_maps.append(m)
    r2 = _run(_cache[key], l2_maps)

    # ---- host permutation of partials back to token order (movement only)
    off_e = np.zeros(E + 1, np.int64)
    for e in range(E):
        off_e[e + 1] = off_e[e] + caps[e]
    pos = np.zeros((S, E), np.int64)
    for e in range(E):
        pos[idx_e[e], e] = off_e[e] + np.arange(len(idx_e[e]))
    colidx = np.take_along_axis(pos, top2, axis=1)           # [S, 2]

    if "l3" not in _cache:
        _cache["l3"] = build_l3()
    # bigs[c]: [128, DC, sum(caps)] concatenated over experts in slot space
    bigs = [np.concatenate([r2[c][f"y{e}"].reshape(128, DC, caps[e])
                            for e in range(E)], axis=2) for c in CORE_IDS]
    l3_maps = []
    for k in CORE_IDS:
        sl = slice(k * SH, (k + 1) * SH)
        x2_p = np.ascontiguousarray(r1[k]["x2T"])
        m = {"x2T": x2_p, "g": np.asarray(g_post_moe, np.float32),
             "ones": ones}
        for c in CORE_IDS:
            for r in range(TOPK):
                m[f"p{c * TOPK + r}"] = np.ascontiguousarray(
                    bigs[c][:, :, colidx[sl, r]].reshape(128, DC * SH))
        l3_maps.append(m)
    r3 = _run(_cache["l3"], l3_maps)

    x3T = np.concatenate([unpack(r3[k]["x3T"], DC) for k in CORE_IDS], axis=1)
    x_out = np.ascontiguousarray(x3T.T)[None]                # [1, S, D]
    return x_out, probs[None]                                # ([1,S,D], [1,S,E])


# revision 19
# speedup vs baseline: 1.0727x; 1.0727x over previous
"""Trainium2 Bass kernel for nn_DecoderLayer_44263932953096 (MQA + top-2 MoE).

Strategy (8 NeuronCores, SPMD via run_bass_kernel_spmd, no on-device collectives):
  L1: token-sharded attention + router. Each core computes shared K/V from the
      full sequence, attention for its 256-token shard, post-attn residual +
      rmsnorms and router softmax. Activations flow feature-on-partition
      ("T layout") so every matmul contracts over the partition dim.
  host: top-2 expert selection (integer compare/indexing only), gathers each
      expert's tokens into per-expert batches.
  L2: expert FFN, sliced along the hidden (FH) axis: core c holds the 512-wide
      FH slice c of ALL 8 experts' W1/W2 and processes every expert's token
      batch for its slice. Perfectly load-balanced against routing skew; one
      uniform NEFF (strip token-caps baked per compile, keyed by counts).
  host: permutes per-slice partial outputs back to token order (movement only).
  L3: token-sharded combine: sums the 16 partials (8 FH slices x 2 experts)
      per token, final rmsnorm + residual.

All floating-point arithmetic runs on device; the host only transposes,
slices, gathers, packs and permutes. Matmuls run as float32r (full-rate fp32).
Every DRAM tensor is host-packed into the exact [128, nchunk*width] big-tile
SBUF layout (chunk-major columns) so each load/store is one long-run 2D DMA.
"""

from contextlib import ExitStack

import numpy as np

import concourse.bacc as bacc
import concourse.mybir as mybir
from concourse import tile
from concourse.bass_utils import run_bass_kernel_spmd

F32 = mybir.dt.float32
F32R = mybir.dt.float32r
AF = mybir.ActivationFunctionType
OP = mybir.AluOpType
AX = mybir.AxisListType

S, D, H, HD, E, TOPK, FH = 2048, 1024, 16, 64, 8, 2, 4096
NCORES = 8
SH = S // NCORES          # 256 tokens per shard
DC = D // 128             # 8 feature chunks
FSLICE = FH // NCORES     # 512 hidden units per core slice
EPS = 1e-5
CORE_IDS = list(range(NCORES))

_cache = {}
L2_BF16 = False


# ---------------------------------------------------------------- helpers

def _nc():
    return bacc.Bacc("TRN2", target_bir_lowering=False, debug=False,
                     num_devices=NCORES)


def _chunks(n, lim=512):
    assert n % 2 == 0
    k = (n + lim - 1) // lim
    base = (n // k) & ~1
    sizes = [base] * k
    deficit = n - base * k
    i = 0
    while deficit > 0:
        sizes[i] += 2; deficit -= 2; i = (i + 1) % k
    return sizes


def pack(m):
    """[nc*128, w] -> big-tile layout [128, nc*w] (chunk-major columns)."""
    nc_, w = m.shape[0] // 128, m.shape[1]
    return np.ascontiguousarray(
        m.reshape(nc_, 128, w).transpose(1, 0, 2).reshape(128, nc_ * w))


def unpack(p, nchunk):
    """inverse of pack: [128, nc*w] -> [nc*128, w]."""
    w = p.shape[1] // nchunk
    return np.ascontiguousarray(
        p.reshape(128, nchunk, w).transpose(1, 0, 2).reshape(nchunk * 128, w))


def _rmsnorm_T(nc, ps_small, big_in, g_big, ones_k, eps_t, N, pool, tag,
               out_bufs=1, out_dt=F32R):
    """T-layout rmsnorm on a big tile [128, DC*N] -> big tile [128, DC*N]:
    out = in * rsqrt(mean_over_D(in^2) + eps) * g.
    g_big is a [128, DC]-shaped view (column kc = gamma chunk kc)."""
    inv = pool.tile([1, N], F32, name=f"{tag}_inv", tag=f"{tag}_inv", bufs=2)
    off = 0
    chm = _chunks(N)[0]
    for ch in _chunks(N):
        pss = ps_small.tile([1, 512], F32, name="colsum", tag="small", bufs=2)
        sq = pool.tile([128, DC * chm], F32R, name=f"{tag}_sq", tag=f"{tag}_sq",
                       bufs=2)
        for kc in range(DC):
            s = sq[:, kc * chm:kc * chm + ch]
            nc.scalar.square(s, big_in[:, kc * N + off:kc * N + off + ch])
            nc.tensor.matmul(pss[:, :ch], ones_k[:], s,
                             start=(kc == 0), stop=(kc == DC - 1))
        nc.scalar.activation(inv[:, off:off + ch], pss[:, :ch], AF.Sqrt,
                             bias=eps_t[:], scale=1.0 / D)
        off += ch
    nc.vector.reciprocal(inv[:], inv[:])
    inv_b = pool.tile([128, N], F32, name=f"{tag}_invb", tag=f"{tag}_invb",
                      bufs=2)
    nc.gpsimd.partition_broadcast(inv_b[:], inv[:])
    out = pool.tile([128, DC * N], out_dt, name=f"{tag}_o", tag=f"{tag}_o",
                    bufs=out_bufs)
    for kc in range(DC):
        nc.vector.scalar_tensor_tensor(out[:, kc * N:(kc + 1) * N],
                                       big_in[:, kc * N:(kc + 1) * N],
                                       g_big[:, kc:kc + 1], inv_b[:],
                                       OP.mult, OP.mult)
    return out


# ---------------------------------------------------------------- L1

def build_l1():
    nc = _nc()
    ap = {}
    def din(name, shape):
        ap[name] = nc.dram_tensor(name, shape, F32, kind="ExternalInput").ap()
    def dout(name, shape):
        ap[name] = nc.dram_tensor(name, shape, F32, kind="ExternalOutput").ap()

    NB = S // 512   # 4 column blocks for the full-sequence pass
    GH = 2          # heads per attention group
    NG = H // GH    # 8 groups
    KC16 = S // 128 # 16 key chunks

    din("xTb", [128, NB * DC * 512])   # packed (block, chunk, col)
    din("xTs", [128, DC * SH])
    din("Wq", [128, DC * D]); din("Wkv", [128, DC * 128])
    din("Wo", [128, DC * D]); din("Wr", [128, DC * E])
    din("gs", [128, DC * 3])
    din("ones", [128]); din("ident", [128, 128])
    dout("x2T", [128, DC * SH]); dout("h2T", [128, DC * SH])
    dout("probs", [SH, E])

    with tile.TileContext(nc) as tc, ExitStack() as ctx:
        keep = ctx.enter_context(tc.tile_pool(name="keep", bufs=1))
        ps_mm = ctx.enter_context(tc.tile_pool(name="ps_mm", bufs=2, space="PSUM"))
        ps_sc = ctx.enter_context(tc.tile_pool(name="ps_sc", bufs=2, space="PSUM"))
        ps_at = ctx.enter_context(tc.tile_pool(name="ps_at", bufs=2, space="PSUM"))
        ps_sm = ctx.enter_context(tc.tile_pool(name="ps_sm", bufs=2, space="PSUM"))

        ones_k = keep.tile([128, 1], F32R, name="ones_k", tag="ones_k")
        nc.sync.dma_start(ones_k[:], ap["ones"].rearrange("(a b) -> a b", b=1).bitcast(F32R))
        eps_t = keep.tile([1, 1], F32, name="eps_t", tag="eps_t")
        nc.vector.memset(eps_t[:], EPS)
        ident = keep.tile([128, 128], F32R, name="ident", tag="ident")
        nc.sync.dma_start(ident[:], ap["ident"].bitcast(F32R))
        gs = keep.tile([128, DC * 3], F32, name="gs", tag="gs")
        nc.sync.dma_start(gs[:], ap["gs"])

        def g_col(which):  # 0=att, 1=post, 2=moe -> [128, DC] strided view
            return gs[:, which::3]

        wr = keep.tile([128, DC * E], F32R, name="wr", tag="wr")
        nc.sync.dma_start(wr[:], ap["Wr"].bitcast(F32R))

        kvT = keep.tile([128, S], F32R, name="kvT", tag="kvT")
        xs = keep.tile([128, DC * SH], F32, name="xs", tag="xs")
        nc.sync.dma_start(xs[:], ap["xTs"])
        att = keep.tile([128, DC * SH], F32R, name="att", tag="att")
        qp = [keep.tile([64, 2 * SH], F32R, name=f"qp{g}", tag=f"qp{g}")
              for g in range(H // 2)]

        with tc.tile_pool(name="stream", bufs=1) as st:
            wq = st.tile([128, DC * D], F32R, name="wq", tag="wq")
            nc.sync.dma_start(wq[:], ap["Wq"].bitcast(F32R))
            wkv = st.tile([128, DC * 128], F32R, name="wkv", tag="wkv")
            nc.sync.dma_start(wkv[:], ap["Wkv"].bitcast(F32R))

            # ---- shard h and per-head qT first (independent of the blocks)
            hs = _rmsnorm_T(nc, ps_sm, xs, g_col(0), ones_k, eps_t, SH, st,
                            tag="sn", out_bufs=2)
            for mc in range(DC):
                pq = ps_mm.tile([128, SH], F32, name="pq", tag="mm")
                for kc in range(DC):
                    nc.tensor.matmul(pq[:], wq[:, kc * D + mc * 128:
                                               kc * D + (mc + 1) * 128],
                                     hs[:, kc * SH:(kc + 1) * SH],
                                     start=(kc == 0), stop=(kc == DC - 1))
                nc.vector.tensor_copy(qp[mc][:, 0:SH], pq[0:64, :])
                nc.vector.tensor_copy(qp[mc][:, SH:2 * SH], pq[64:128, :])

            # ---- full-seq h = rmsnorm(x, g_att), kvT = Wkv.T @ hT
            for nb in range(NB):
                c0 = nb * 512
                xb = st.tile([128, DC * 512], F32, name="xb", tag="xb", bufs=2)
                nc.sync.dma_start(xb[:], ap["xTb"][:, nb * DC * 512:
                                                   (nb + 1) * DC * 512])
                hb = _rmsnorm_T(nc, ps_sm, xb, g_col(0), ones_k, eps_t, 512,
                                st, tag="sn", out_bufs=2)
                pkv = ps_mm.tile([128, 512], F32, name="pkv", tag="mm")
                for kc in range(DC):
                    nc.tensor.matmul(pkv[:], wkv[:, kc * 128:(kc + 1) * 128],
                                     hb[:, kc * 512:(kc + 1) * 512],
                                     start=(kc == 0), stop=(kc == DC - 1))
                nc.vector.tensor_copy(kvT[:, c0:c0 + 512], pkv[:])

        # ---- attention (Wo prefetches concurrently in its own pool)
        wop = ctx.enter_context(tc.tile_pool(name="wop", bufs=1))
        wo = wop.tile([128, DC * D], F32R, name="wo", tag="wo")
        nc.sync.dma_start(wo[:], ap["Wo"].bitcast(F32R))
        with tc.tile_pool(name="attn", bufs=1) as at:
            v_aug = []
            for kc in range(KC16):
                pt = ps_sm.tile([128, 64], F32R, name="pt", tag="small", bufs=2)
                nc.tensor.transpose(pt[:], kvT[64:128, kc * 128:(kc + 1) * 128],
                                    ident[64:128, 0:64])
                va = at.tile([128, 65], F32R, name=f"vaug{kc}", tag=f"vaug{kc}")
                nc.vector.tensor_copy(va[:, 0:64], pt[:])
                nc.vector.tensor_copy(va[:, 64:65], ones_k[:])
                v_aug.append(va)

            for g in range(NG):
                pa = ps_at.tile([65, GH * SH], F32, name="pa", tag="attn")
                for kc in range(KC16):
                    psc = ps_sc.tile([128, GH * SH], F32, name="psc", tag="sc")
                    nc.tensor.matmul(psc[:], kvT[0:64, kc * 128:(kc + 1) * 128],
                                     qp[g][:], start=True, stop=True)
                    ex = at.tile([128, GH * SH], F32R, name="ex", tag="exp",
                                 bufs=16)
                    nc.scalar.activation(ex[:], psc[:], AF.Exp, scale=0.125)
                    nc.tensor.matmul(pa[:], v_aug[kc][:], ex[:],
                                     start=(kc == 0), stop=(kc == KC16 - 1))
                rec = at.tile([1, GH * SH], F32, name="rec", tag="rec", bufs=2)
                nc.vector.reciprocal(rec[:], pa[64:65, :])
                for j in range(GH):
                    h = g * GH + j
                    rb = at.tile([64, SH], F32, name="rb", tag="rb", bufs=2)
                    nc.gpsimd.partition_broadcast(rb[:], rec[:, j * SH:(j + 1) * SH])
                    dst = att[64 * (h % 2):64 * (h % 2) + 64,
                              (h // 2) * SH:(h // 2 + 1) * SH]
                    nc.vector.tensor_tensor(dst, pa[0:64, j * SH:(j + 1) * SH],
                                            rb[:], OP.mult)

        # ---- tail: out-proj, norms, residual, router
        with tc.tile_pool(name="tail", bufs=1) as tl:
            ao = tl.tile([128, DC * SH], F32, name="ao", tag="ao")
            for mc in range(DC):
                po = ps_mm.tile([128, SH], F32, name="po", tag="mm")
                for kc in range(DC):
                    nc.tensor.matmul(po[:], wo[:, kc * D + mc * 128:
                                               kc * D + (mc + 1) * 128],
                                     att[:, kc * SH:(kc + 1) * SH],
                                     start=(kc == 0), stop=(kc == DC - 1))
                nc.vector.tensor_copy(ao[:, mc * SH:(mc + 1) * SH], po[:])

            aon = _rmsnorm_T(nc, ps_sm, ao, g_col(1), ones_k, eps_t, SH, tl,
                             tag="aon", out_dt=F32)
            x2 = tl.tile([128, DC * SH], F32, name="x2", tag="x2")
            for kc in range(DC):
                nc.vector.tensor_tensor(x2[:, kc * SH:(kc + 1) * SH],
                                        xs[:, kc * SH:(kc + 1) * SH],
                                        aon[:, kc * SH:(kc + 1) * SH], OP.add)
            nc.sync.dma_start(ap["x2T"], x2[:])
            h2 = _rmsnorm_T(nc, ps_sm, x2, g_col(2), ones_k, eps_t, SH, tl,
                            tag="h2")
            nc.sync.dma_start(ap["h2T"].bitcast(F32R), h2[:])

            for t2 in range(SH // 128):
                pl = ps_sm.tile([128, E], F32, name="pl", tag="small", bufs=2)
                for kc in range(DC):
                    nc.tensor.matmul(pl[:], h2[:, kc * SH + t2 * 128:
                                               kc * SH + (t2 + 1) * 128],
                                     wr[:, kc * E:(kc + 1) * E],
                                     start=(kc == 0), stop=(kc == DC - 1))
                mx = tl.tile([128, 1], F32, name="mx", tag="mx", bufs=2)
                nc.vector.reduce_max(mx[:], pl[:], axis=AX.X)
                shl = tl.tile([128, E], F32, name="shl", tag="shl", bufs=2)
                nc.vector.tensor_scalar(shl[:], pl[:], mx[:], None, OP.subtract)
                exl = tl.tile([128, E], F32, name="exl", tag="exl", bufs=2)
                sm = tl.tile([128, 1], F32, name="sm", tag="sm", bufs=2)
                nc.scalar.activation(exl[:], shl[:], AF.Exp, accum_out=sm[:])
                nc.vector.reciprocal(sm[:], sm[:])
                pr = tl.tile([128, E], F32, name="pr", tag="pr", bufs=2)
                nc.vector.tensor_scalar(pr[:], exl[:], sm[:], None, OP.mult)
                nc.sync.dma_start(ap["probs"][t2 * 128:(t2 + 1) * 128, :], pr[:])

    nc.compile()
    return nc


# ---------------------------------------------------------------- L2

def build_l2(caps, bf16=False):
    nc = _nc()
    idt = mybir.dt.bfloat16 if bf16 else F32
    mdt = mybir.dt.bfloat16 if bf16 else F32R
    ap = {}
    for e in range(E):
        c = caps[e]
        ap[f"hg{e}"] = nc.dram_tensor(f"hg{e}", [128, DC * c], idt, kind="ExternalInput").ap()
        ap[f"w1_{e}"] = nc.dram_tensor(f"w1_{e}", [128, DC * FSLICE], idt, kind="ExternalInput").ap()
        ap[f"w2_{e}"] = nc.dram_tensor(f"w2_{e}", [128, (FSLICE // 128) * D], idt, kind="ExternalInput").ap()
        ap[f"wv{e}"] = nc.dram_tensor(f"wv{e}", [1, c], F32, kind="ExternalInput").ap()
        ap[f"y{e}"] = nc.dram_tensor(f"y{e}", [128, DC * c], F32, kind="ExternalOutput").ap()

    MC1 = FSLICE // 128   # 4
    with tile.TileContext(nc) as tc:
        with tc.tile_pool(name="hg", bufs=2) as hgp, \
             tc.tile_pool(name="w", bufs=1) as wp, \
             tc.tile_pool(name="hid", bufs=2) as hp, \
             tc.tile_pool(name="out", bufs=2) as op_, \
             tc.tile_pool(name="ps", bufs=8, space="PSUM") as ps:

            for e in range(E):
                cap = caps[e]
                nch = _chunks(cap)
                w1 = wp.tile([128, DC * FSLICE], mdt, name="w1", tag="w1")
                nc.sync.dma_start(w1[:], ap[f"w1_{e}"] if bf16 else ap[f"w1_{e}"].bitcast(F32R))
                hg = hgp.tile([128, DC * cap], mdt, name="hg", tag="hg")
                for q in range(4):   # split load: 2 feature chunks per DMA
                    lo, hi = q * 2 * cap, (q + 1) * 2 * cap
                    nc.sync.dma_start(hg[:, lo:hi], (ap[f"hg{e}"] if bf16 else
                                      ap[f"hg{e}"].bitcast(F32R))[:, lo:hi])

                hid = hp.tile([128, MC1 * cap], mdt, name="hid", tag="hid")
                w2 = wp.tile([128, MC1 * D], mdt, name="w2", tag="w2")
                wvs = op_.tile([1, cap], F32, name="wvs", tag="wvs")
                wb = op_.tile([128, cap], F32, name="wb", tag="wb")
                yt = op_.tile([128, DC * cap], F32, name="yt", tag="yt", bufs=1)
                w2_loaded = False
                off = 0
                for ci, ch in enumerate(nch):
                    for mc in range(MC1):
                        p = ps.tile([128, 512], F32, name="p1", tag="ps")
                        for kc in range(DC):
                            nc.tensor.matmul(
                                p[:, :ch],
                                w1[:, kc * FSLICE + mc * 128:
                                   kc * FSLICE + (mc + 1) * 128],
                                hg[:, kc * cap + off:kc * cap + off + ch],
                                start=(kc == 0), stop=(kc == DC - 1))
                        nc.scalar.activation(hid[:, mc * cap + off:
                                                 mc * cap + off + ch],
                                             p[:, :ch], AF.Gelu_apprx_tanh)
                    if not w2_loaded:
                        # defer the FFN2 operand loads until FFN1 is underway
                        nc.sync.dma_start(w2[:], ap[f"w2_{e}"] if bf16 else
                                          ap[f"w2_{e}"].bitcast(F32R))
                        nc.sync.dma_start(wvs[:], ap[f"wv{e}"])
                        nc.gpsimd.partition_broadcast(wb[:], wvs[:])
                        w2_loaded = True
                    for mc in range(DC):
                        p = ps.tile([128, 512], F32, name="p2", tag="ps")
                        for kc in range(MC1):
                            nc.tensor.matmul(
                                p[:, :ch],
                                w2[:, kc * D + mc * 128:kc * D + (mc + 1) * 128],
                                hid[:, kc * cap + off:kc * cap + off + ch],
                                start=(kc == 0), stop=(kc == MC1 - 1))
                        nc.vector.tensor_tensor(yt[:, mc * cap + off:
                                                   mc * cap + off + ch],
                                                p[:, :ch],
                                                wb[:, off:off + ch], OP.mult)
                    off += ch
                for q in range(4):   # split store: 2 feature chunks per DMA
                    lo, hi = q * 2 * cap, (q + 1) * 2 * cap
                    nc.sync.dma_start(ap[f"y{e}"][:, lo:hi], yt[:, lo:hi])

    nc.compile()
    return nc


# ---------------------------------------------------------------- L3

NPART = NCORES * TOPK  # 16 partial inputs per token

def build_l3():
    nc = _nc()
    ap = {}
    for p in range(NPART):
        ap[f"p{p}"] = nc.dram_tensor(f"p{p}", [128, DC * SH], F32, kind="ExternalInput").ap()
    ap["x2T"] = nc.dram_tensor("x2T", [128, DC * SH], F32, kind="ExternalInput").ap()
    ap["g"] = nc.dram_tensor("g", [D], F32, kind="ExternalInput").ap()
    ap["ones"] = nc.dram_tensor("ones", [128], F32, kind="ExternalInput").ap()
    ap["x3T"] = nc.dram_tensor("x3T", [128, DC * SH], F32, kind="ExternalOutput").ap()

    with tile.TileContext(nc) as tc:
        with tc.tile_pool(name="sb", bufs=1) as sb, \
             tc.tile_pool(name="st", bufs=1) as st, \
             tc.tile_pool(name="ps", bufs=2, space="PSUM") as ps1:
            ones_k = sb.tile([128, 1], F32R, name="ones_k", tag="ones_k")
            nc.sync.dma_start(ones_k[:], ap["ones"].rearrange("(a b) -> a b", b=1).bitcast(F32R))
            eps_t = sb.tile([1, 1], F32, name="eps_t", tag="eps_t")
            nc.vector.memset(eps_t[:], EPS)
            g_big = sb.tile([128, DC], F32, name="g_big", tag="g_big")
            nc.sync.dma_start(g_big[:], ap["g"].rearrange("(c p) -> p c", p=128))
            x2_t = sb.tile([128, DC * SH], F32, name="x2t", tag="x2t")
            nc.sync.dma_start(x2_t[:], ap["x2T"])

            pins = []
            for p in range(NPART):
                t = st.tile([128, DC * SH], F32, name=f"pin{p}", tag="pin",
                            bufs=6)
                nc.sync.dma_start(t[:], ap[f"p{p}"])
                pins.append(t)
            lvl = pins
            li = 0
            while len(lvl) > 1:
                nxt = []
                for i in range(0, len(lvl), 2):
                    o = st.tile([128, DC * SH], F32, name=f"sum{li}_{i}",
                                tag=f"sum{li}", bufs=(2 if len(lvl) > 2 else 1))
                    nc.vector.tensor_tensor(o[:], lvl[i][:], lvl[i + 1][:],
                                            OP.add)
                    nxt.append(o)
                lvl = nxt
                li += 1
            mo = lvl[0]

            mon = _rmsnorm_T(nc, ps1, mo, g_big, ones_k, eps_t, SH, sb,
                             tag="mon", out_dt=F32)
            x3 = sb.tile([128, DC * SH], F32, name="x3", tag="x3")
            for kc in range(DC):
                nc.vector.tensor_tensor(x3[:, kc * SH:(kc + 1) * SH],
                                        x2_t[:, kc * SH:(kc + 1) * SH],
                                        mon[:, kc * SH:(kc + 1) * SH], OP.add)
            nc.sync.dma_start(ap["x3T"], x3[:])

    nc.compile()
    return nc


# ---------------------------------------------------------------- host

def _run(nc, in_maps):
    return run_bass_kernel_spmd(nc, in_maps, CORE_IDS).results


def kernel(x, Wq, Wk, Wv, Wo, g_pre_mqa, g_post_mqa, g_pre_moe, g_post_moe,
           Wr, W1, W2):
    x = np.ascontiguousarray(np.asarray(x, dtype=np.float32))
    xT = np.ascontiguousarray(x[0].T)                       # [D, S]
    # xTb: packed (block, chunk, col) so each 512-col block is contiguous
    xTb = np.ascontiguousarray(
        xT.reshape(DC, 128, S // 512, 512).transpose(1, 2, 0, 3)
          .reshape(128, (S // 512) * DC * 512))
    Wkv = np.concatenate([np.asarray(Wk), np.asarray(Wv)], axis=1).astype(np.float32)
    ones = np.ones(128, np.float32)
    ident = np.zeros((128, 128), np.float32)
    ident[0:64, 0:64] = np.eye(64)
    ident[64:128, 0:64] = np.eye(64)
    gs = np.stack([np.asarray(g_pre_mqa), np.asarray(g_post_mqa),
                   np.asarray(g_pre_moe)], axis=1).astype(np.float32)  # [D,3]

    if "l1" not in _cache:
        _cache["l1"] = build_l1()
    wq_p = pack(np.asarray(Wq, np.float32))
    wkv_p = pack(Wkv)
    wo_p = pack(np.asarray(Wo, np.float32))
    wr_p = pack(np.asarray(Wr, np.float32))
    gs_p = pack(gs)
    xT_p = pack(xT)                                          # [128, DC*S]
    l1_maps = []
    for c in CORE_IDS:
        xs_p = np.ascontiguousarray(
            xT_p.reshape(128, DC, S)[:, :, c * SH:(c + 1) * SH]
                .reshape(128, DC * SH))
        l1_maps.append({
            "xTb": xTb, "xTs": xs_p, "Wq": wq_p, "Wkv": wkv_p,
            "Wo": wo_p, "Wr": wr_p, "gs": gs_p, "ones": ones, "ident": ident,
        })
    r1 = _run(_cache["l1"], l1_maps)
    # packed per-shard [128, DC*SH] -> h2 full matrix [D, S] for gathering
    h2_pk = np.stack([r1[c]["h2T"].reshape(128, DC, SH) for c in CORE_IDS])
    h2_full = np.ascontiguousarray(
        h2_pk.transpose(1, 2, 0, 3).reshape(128, DC, S))     # [128, DC, S]
    probs = np.concatenate([r1[c]["probs"] for c in CORE_IDS], axis=0)  # [S, E]

    # ---- host routing bookkeeping (indexing only)
    order = np.argsort(-probs, axis=1, kind="stable")
    top2 = order[:, :TOPK]                                   # [S, 2]
    idx_e = [np.where((top2 == e).any(axis=1))[0] for e in range(E)]
    counts = [len(ix) for ix in idx_e]
    caps = tuple(max(256, -(-cnt // 64) * 64) for cnt in counts)

    key = ("l2", caps, L2_BF16)
    if key not in _cache:
        _cache[key] = build_l2(caps, bf16=L2_BF16)

    W1 = np.asarray(W1, np.float32); W2 = np.asarray(W2, np.float32)
    if L2_BF16:
        import ml_dtypes
        idt = ml_dtypes.bfloat16
    else:
        idt = np.float32
    l2_shared = {}
    for e in range(E):
        cap, ix = caps[e], idx_e[e]
        hg = np.zeros((128, DC, cap), np.float32)
        hg[:, :, :len(ix)] = h2_full[:, :, ix]
        l2_shared[f"hg{e}"] = hg.reshape(128, DC * cap).astype(idt)
        wv = np.zeros((1, cap), np.float32)
        wv[0, :len(ix)] = probs[ix, e]
        l2_shared[f"wv{e}"] = wv
    l2_maps = []
    for c in CORE_IDS:
        m = dict(l2_shared)
        for e in range(E):
            m[f"w1_{e}"] = pack(np.ascontiguousarray(
                W1[e][:, c * FSLICE:(c + 1) * FSLICE])).astype(idt)
            m[f"w2_{e}"] = pack(np.ascontiguousarray(
                W2[e][c * FSLICE:(c + 1) * FSLICE, :])).astype(idt)
        l2_maps.append(m)
    r2 = _run(_cache[key], l2_maps)

    # ---- host permutation of partials back to token order (movement only)
    off_e = np.zeros(E + 1, np.int64)
    for e in range(E):
        off_e[e + 1] = off_e[e] + caps[e]
    pos = np.zeros((S, E), np.int64)
    for e in range(E):
        pos[idx_e[e], e] = off_e[e] + np.arange(len(idx_e[e]))
    colidx = np.take_along_axis(pos, top2, axis=1)           # [S, 2]

    if "l3" not in _cache:
        _cache["l3"] = build_l3()
    # bigs[c]: [128, DC, sum(caps)] concatenated over experts in slot space
    bigs = [np.concatenate([r2[c][f"y{e}"].reshape(128, DC, caps[e])
                            for e in range(E)], axis=2) for c in CORE_IDS]
    l3_maps = []
    for k in CORE_IDS:
        sl = slice(k * SH, (k + 1) * SH)
        x2_p = np.ascontiguousarray(r1[k]["x2T"])
        m = {"x2T": x2_p, "g": np.asarray(g_post_moe, np.float32),
             "ones": ones}
        for c in CORE_IDS:
            for r in range(TOPK):
                m[f"p{c * TOPK + r}"] = np.ascontiguousarray(
                    bigs[c][:, :, colidx[sl, r]].reshape(128, DC * SH))
        l3_maps.append(m)
    r3 = _run(_cache["l3"], l3_maps)

    x3T = np.concatenate([unpack(r3[k]["x3T"], DC) for k in CORE_IDS], axis=1)
    x_out = np.ascontiguousarray(x3T.T)[None]                # [1, S, D]
    return x_out, probs[None]                                # ([1,S,D], [1,S,E])
